# revision 15
# baseline (speedup 1.0000x reference)
"""Trainium2 Bass kernel for nn_MultiHeadAttention_45921790329378.

Full (unsharded) inputs in, full outputs back. Internally shards the
batch dimension across 8 NeuronCores (B=8 -> one batch per core, all 8
heads per core); weights replicated.

Per-core dataflow (single NeuronCore, Tile framework, fp32r matmuls):
  phase 1: layernorm q/k/v (bn_stats, batched rsqrt; LN gain folded into
           projection weights host-side), PE-transpose normalized
           activations to put d_in on partitions.
  phase 1b: projections (fp32r). qhT/khT/vhT in [d, l] head-pair tiles
           (head 2p rows 0-63, head 2p+1 rows 64-127), vh in [l, d]
           with an appended ones column for the softmax denominator.
  phase 2: per head-pair, per q-half: scoresT[k, q] via K=64 fp32r
           matmuls on disjoint PE row groups (concurrent); diagonal mask
           injected into the PSUM accumulation with a bf16 I.T @
           (-1e32*I) matmul; exp on ACT straight from PSUM; PV matmuls
           with lhsT=[vh|1] yield O^T and the denominator row Z; Zinv is
           partition-broadcast with a K=1 matmul, and the exp tiles are
           normalized elementwise (DVE + GPSIMD) and written out as
           attn^T.
  phase 3: dynamic = O @ w_fc1, static = vh_all @ w_fc2 from the [hd, l]
           operands produced above.

attn leaves the device as [h, k, q] per batch; the host transposes to
attn_flat[h*B+b, q, k] (pure layout rearrangement of device values).
"""

import numpy as np

import concourse.bacc as bacc
import concourse.mybir as mybir
import concourse.tile as tile
from concourse import bass_utils

N_CORES = 8
B, L, D_IN = 8, 1024, 512
N_HEAD, D_K, D_V, D_MODEL = 8, 64, 64, 512
LC = L // 128          # 8 l-chunks of 128
KC = D_IN // 128       # 4 d_in-chunks of 128
NEG_BIG = -1e32
EPS = 1e-5

F32 = mybir.dt.float32
F32R = mybir.dt.float32r
BF16 = mybir.dt.bfloat16
AF = mybir.ActivationFunctionType
ALU = mybir.AluOpType

# of the 8 per-pair attn-normalize multiplies, how many on gpsimd
ATTN_TT_ON_GPSIMD = 2

_CACHED = {}


def _build_nc():
    nc = bacc.Bacc("TRN2", target_bir_lowering=False, debug=False)

    xq = nc.dram_tensor("xq", [L, D_IN], F32, kind="ExternalInput")
    xk = nc.dram_tensor("xk", [L, D_IN], F32, kind="ExternalInput")
    xv = nc.dram_tensor("xv", [L, D_IN], F32, kind="ExternalInput")
    wq = nc.dram_tensor("wq", [D_IN, D_IN], F32R, kind="ExternalInput")
    wk = nc.dram_tensor("wk", [D_IN, D_IN], F32R, kind="ExternalInput")
    wv = nc.dram_tensor("wv", [D_IN, D_IN], F32R, kind="ExternalInput")
    w1 = nc.dram_tensor("w1", [D_IN, D_MODEL], F32R, kind="ExternalInput")
    w2 = nc.dram_tensor("w2", [D_IN, D_MODEL], F32R, kind="ExternalInput")
    eyer = nc.dram_tensor("eyer", [128, 128], F32R, kind="ExternalInput")
    eyebp = nc.dram_tensor("eyebp", [128, 128], BF16, kind="ExternalInput")
    eyebn = nc.dram_tensor("eyebn", [128, 128], BF16, kind="ExternalInput")
    onesr = nc.dram_tensor("onesr", [1, 128], F32R, kind="ExternalInput")
    onescol = nc.dram_tensor("onescol", [128, N_HEAD], F32R, kind="ExternalInput")

    attn_t = nc.dram_tensor("attn_t", [N_HEAD, L, L], F32, kind="ExternalOutput")
    dyn = nc.dram_tensor("dyn", [L, D_MODEL], F32, kind="ExternalOutput")
    stat = nc.dram_tensor("stat", [L, D_MODEL], F32, kind="ExternalOutput")

    with tile.TileContext(nc) as tc:
        _emit(nc, tc, xq, xk, xv, wq, wk, wv, w1, w2, eyer, eyebp, eyebn,
              onesr, onescol, attn_t, dyn, stat)
    nc.compile()
    return nc


def _emit(nc, tc, xq, xk, xv, wq, wk, wv, w1, w2, eyer, eyebp, eyebn,
          onesr, onescol, attn_t, dyn, stat):
    from contextlib import ExitStack
    ctx = ExitStack()
    with ctx:
        consts = ctx.enter_context(tc.tile_pool(name="consts", bufs=1))
        w1_sb = [consts.tile([128, D_MODEL], F32R, name=f"w1_{kc}") for kc in range(KC)]
        w2_sb = [consts.tile([128, D_MODEL], F32R, name=f"w2_{kc}") for kc in range(KC)]
        for kc in range(KC):
            nc.sync.dma_start(out=w1_sb[kc], in_=w1.ap()[kc * 128:(kc + 1) * 128, :])
            nc.sync.dma_start(out=w2_sb[kc], in_=w2.ap()[kc * 128:(kc + 1) * 128, :])
        eyer_sb = consts.tile([128, 128], F32R, name="eyer_sb")
        eyebp_sb = consts.tile([128, 128], BF16, name="eyebp_sb")
        eyebn_sb = consts.tile([128, 128], BF16, name="eyebn_sb")
        nc.sync.dma_start(out=eyer_sb, in_=eyer.ap())
        nc.sync.dma_start(out=eyebp_sb, in_=eyebp.ap())
        nc.sync.dma_start(out=eyebn_sb, in_=eyebn.ap())
        ones1 = consts.tile([1, 128], F32R, name="ones1")
        nc.sync.dma_start(out=ones1, in_=onesr.ap())
        eps_sb = consts.tile([128, 1], F32, name="eps_sb")
        nc.vector.memset(eps_sb, EPS)

        # persistent activation layouts (fp32r)
        persist = ctx.enter_context(tc.tile_pool(name="persist", bufs=1))
        qhT = [persist.tile([128, L], F32R, name=f"qhT_{p}") for p in range(4)]
        khT = [persist.tile([128, L], F32R, name=f"khT_{p}") for p in range(4)]
        vhT = [persist.tile([128, L], F32R, name=f"vhT_{p}") for p in range(4)]
        vhp = [persist.tile([128, N_HEAD, D_V + 1], F32R, name=f"vhp_{c}")
               for c in range(LC)]
        otp = [persist.tile([128, L], F32R, name=f"otp_{p}") for p in range(4)]

        # ---------------- phase 1: LN + transpose + projections ----------
        with tc.tile_pool(name="p1_w", bufs=1) as p1_w, \
             tc.tile_pool(name="p1_xnt", bufs=1) as p1_xnt, \
             tc.tile_pool(name="p1_x", bufs=9) as p1_x, \
             tc.tile_pool(name="p1_sb", bufs=3) as p1_sb, \
             tc.tile_pool(name="p1_st", bufs=2) as p1_st, \
             tc.tile_pool(name="p1_ps_tr", bufs=4, space="PSUM") as p1_ps_tr, \
             tc.tile_pool(name="p1_ps_pj", bufs=3, space="PSUM") as p1_ps_pj:

            wq_sb = [p1_w.tile([128, D_IN], F32R, name=f"wq_{kc}") for kc in range(KC)]
            wk_sb = [p1_w.tile([128, D_IN], F32R, name=f"wk_{kc}") for kc in range(KC)]
            wv_sb = [p1_w.tile([128, D_IN], F32R, name=f"wv_{kc}") for kc in range(KC)]
            for kc in range(KC):
                nc.sync.dma_start(out=wq_sb[kc], in_=wq.ap()[kc * 128:(kc + 1) * 128, :])
                nc.sync.dma_start(out=wk_sb[kc], in_=wk.ap()[kc * 128:(kc + 1) * 128, :])
                nc.sync.dma_start(out=wv_sb[kc], in_=wv.ap()[kc * 128:(kc + 1) * 128, :])

            xnT = {}
            for t in range(3):
                for kc in range(KC):
                    xnT[(t, kc)] = p1_xnt.tile([128, L], F32R, name=f"xnT_{t}_{kc}")

            for t, x_in in enumerate([xq, xk, xv]):
                xts = []
                mvall = p1_st.tile([128, 2 * LC], F32, name="mvall", tag="mvall")
                for c in range(LC):
                    xt = p1_x.tile([128, D_IN], F32, name="xt", tag="xt")
                    nc.sync.dma_start(out=xt, in_=x_in.ap()[c * 128:(c + 1) * 128, :])
                    xts.append(xt)
                    stt = p1_sb.tile([128, 6], F32, name="stt", tag="stt")
                    nc.vector.bn_stats(out=stt, in_=xt)
                    nc.vector.bn_aggr(out=mvall[:, 2 * c:2 * c + 2], in_=stt)
                # batched rstd: sig = sqrt(var + eps); rs = 1/sig
                sg = p1_st.tile([128, LC], F32, name="sg", tag="sg")
                var_view = mvall.rearrange("p (c two) -> p c two", two=2)[:, :, 1]
                nc.scalar.activation(sg, var_view, AF.Sqrt, bias=eps_sb)
                rs = p1_st.tile([128, LC], F32, name="rs", tag="rs")
                nc.vector.reciprocal(out=rs, in_=sg)
                ps_tr = {}
                for c in range(LC):
                    xn = p1_sb.tile([128, D_IN], F32R, name="xn", tag="xn")
                    nc.vector.tensor_scalar(
                        out=xn, in0=xts[c], scalar1=mvall[:, 2 * c:2 * c + 1],
                        scalar2=rs[:, c:c + 1],
                        op0=ALU.subtract, op1=ALU.mult)
                    half = c // 4
                    for dc in range(KC):
                        key = (dc, half)
                        if key not in ps_tr:
                            ps_tr[key] = p1_ps_tr.tile([128, 512], F32R,
                                                       name="ps_tr", tag="ps_tr")
                        cc = c % 4
                        nc.tensor.transpose(
                            ps_tr[key][:, cc * 128:(cc + 1) * 128],
                            xn[:, dc * 128:(dc + 1) * 128], eyer_sb)
                        if cc == 3:
                            nc.scalar.copy(
                                out=xnT[(t, dc)][:, half * 512:(half + 1) * 512],
                                in_=ps_tr[key])
                            del ps_tr[key]

            # projections: qhT/khT/vhT head-pair [d, l] tiles (fp32r)
            for src, (wt, dstT) in enumerate(
                    ((wq_sb, qhT), (wk_sb, khT), (wv_sb, vhT))):
                for p in range(4):
                    for lh in range(2):
                        pj = p1_ps_pj.tile([128, 512], F32, name="pj", tag="pj")
                        for kc in range(KC):
                            nc.tensor.matmul(
                                pj, wt[kc][:, p * 128:(p + 1) * 128],
                                xnT[(src, kc)][:, lh * 512:(lh + 1) * 512],
                                start=(kc == 0), stop=(kc == KC - 1))
                        nc.scalar.copy(out=dstT[p][:, lh * 512:(lh + 1) * 512],
                                       in_=pj)
            # vh natural [l, d] with ones column
            for c in range(LC):
                pj = p1_ps_pj.tile([128, 512], F32, name="pjv", tag="pj")
                for kc in range(KC):
                    nc.tensor.matmul(
                        pj, xnT[(2, kc)][:, c * 128:(c + 1) * 128], wv_sb[kc],
                        start=(kc == 0), stop=(kc == KC - 1))
                nc.vector.tensor_copy(
                    out=vhp[c][:, :, 0:D_V],
                    in_=pj.rearrange("p (h d) -> p h d", h=N_HEAD))
                nc.sync.dma_start(out=vhp[c][:, :, D_V], in_=onescol.ap())

        # ---------------- phase 2: attention per head-pair ---------------
        # et tiles are [128 k, 2048] = (qh 2, head-in-pair 2, q' 512)
        # zb tiles share that layout; attn normalization is one in-place
        # [128, 2048] multiply per (pair, k-chunk), then a 1 MB DMA.
        with tc.tile_pool(name="p2_et", bufs=8) as p2_et, \
             tc.tile_pool(name="p2_z", bufs=4) as p2_z, \
             tc.tile_pool(name="p2_zb", bufs=1) as p2_zb, \
             tc.tile_pool(name="ps_s", bufs=2, space="PSUM") as ps_s_pool, \
             tc.tile_pool(name="ps_ot", bufs=1, space="PSUM") as ps_ot_pool, \
             tc.tile_pool(name="ps_zb", bufs=1, space="PSUM") as ps_zb_pool:

            for pr in range(4):
                hA = 2 * pr
                ets = {}
                zb_sb = p2_zb.tile([128, 2 * L], F32, name="zb_sb", tag="zb")
                for qh in range(2):
                    qs = slice(qh * 512, (qh + 1) * 512)
                    ps_ot = ps_ot_pool.tile([D_V + 1, L], F32, name="ps_ot",
                                            tag="ps_ot")
                    for kc in range(LC):
                        ps_s = ps_s_pool.tile([128, L], F32, name="ps_s",
                                              tag="ps_s")
                        diag_here = (qh * 512 <= kc * 128 < (qh + 1) * 512)
                        for hh in range(2):  # head in pair; PE row groups
                            o = 64 * hh
                            nc.tensor.matmul(
                                ps_s[:, hh * 512:(hh + 1) * 512],
                                khT[pr][o:o + 64, kc * 128:(kc + 1) * 128],
                                qhT[pr][o:o + 64, qs],
                                start=True, stop=not diag_here)
                        if diag_here:
                            d0 = kc * 128 - qh * 512
                            for hh in range(2):
                                nc.tensor.matmul(
                                    ps_s[:, hh * 512 + d0:hh * 512 + d0 + 128],
                                    eyebp_sb, eyebn_sb, start=False, stop=True)
                        if qh == 0:
                            ets[kc] = p2_et.tile([128, 2 * L], F32R,
                                                 name="et", tag="et")
                        et = ets[kc]
                        nc.scalar.activation(et[:, qh * L:(qh + 1) * L], ps_s,
                                             AF.Exp)
                        for hh in range(2):
                            nc.tensor.matmul(
                                ps_ot[:, hh * 512:(hh + 1) * 512],
                                vhp[kc][:, hA + hh, :],
                                et[:, qh * L + hh * 512:qh * L + (hh + 1) * 512],
                                start=(kc == 0), stop=(kc == LC - 1))
                    # zinv = exp(-ln Z) on ACT (1-lane rows; avoids the slow
                    # DVE iterative-divide reciprocal)
                    lnz = p2_z.tile([1, L], F32, name="lnz", tag="lnz")
                    nc.scalar.activation(lnz, ps_ot[D_V:D_V + 1, :], AF.Ln)
                    zinv = p2_z.tile([1, L], F32R, name="zinv", tag="zinv")
                    nc.scalar.activation(zinv, lnz, AF.Exp, scale=-1.0)
                    ps_zb = ps_zb_pool.tile([128, L], F32, name="ps_zb",
                                            tag="ps_zb")
                    for hh in range(2):
                        nc.tensor.matmul(ps_zb[:, hh * 512:(hh + 1) * 512],
                                         ones1, zinv[:, hh * 512:(hh + 1) * 512],
                                         start=True, stop=True)
                    nc.scalar.copy(out=zb_sb[:, qh * L:(qh + 1) * L], in_=ps_zb)
                    # O^T rows: head A -> otp rows 0-63, head B -> rows 64-127
                    for hh in range(2):
                        o = 64 * hh
                        nc.scalar.copy(out=otp[pr][o:o + 64, qs],
                                       in_=ps_ot[0:D_V, hh * 512:(hh + 1) * 512])
                        nc.vector.tensor_mul(
                            out=otp[pr][o:o + 64, qs],
                            in0=otp[pr][o:o + 64, qs],
                            in1=zb_sb[o:o + 64, qh * L + hh * 512:
                                      qh * L + (hh + 1) * 512])
                # normalized attn^T -> HBM (1 MB per DMA)
                for kc in range(LC):
                    et = ets[kc]
                    eng = nc.gpsimd if kc < ATTN_TT_ON_GPSIMD else nc.vector
                    with nc.allow_low_precision(reason="fp32r attn tiles"):
                        eng.tensor_mul(out=et, in0=et, in1=zb_sb)
                    dst = attn_t.ap()[hA:hA + 2, kc * 128:(kc + 1) * 128,
                                      :].rearrange("h k (a q) -> k a h q", a=2)
                    nc.sync.dma_start(
                        out=dst,
                        in_=et.bitcast(F32).rearrange("p (a h q) -> p a h q",
                                                      a=2, h=2))

        # ---------------- phase 3: dynamic / static ----------------------
        with tc.tile_pool(name="p3_sb", bufs=4) as p3_sb, \
             tc.tile_pool(name="p3_ps", bufs=4, space="PSUM") as p3_ps:
            for c in range(LC):
                pd = p3_ps.tile([128, D_MODEL], F32, name="pd", tag="pd")
                for p in range(4):
                    nc.tensor.matmul(pd, otp[p][:, c * 128:(c + 1) * 128],
                                     w1_sb[p], start=(p == 0), stop=(p == 3))
                db = p3_sb.tile([128, D_MODEL], F32, name="db", tag="db")
                nc.scalar.copy(out=db, in_=pd)
                nc.sync.dma_start(out=dyn.ap()[c * 128:(c + 1) * 128, :], in_=db)

                ps2 = p3_ps.tile([128, D_MODEL], F32, name="ps2", tag="pd")
                for p in range(4):
                    nc.tensor.matmul(ps2, vhT[p][:, c * 128:(c + 1) * 128],
                                     w2_sb[p], start=(p == 0), stop=(p == 3))
                sb2 = p3_sb.tile([128, D_MODEL], F32, name="sb2", tag="db")
                nc.scalar.copy(out=sb2, in_=ps2)
                nc.sync.dma_start(out=stat.ap()[c * 128:(c + 1) * 128, :], in_=sb2)


def _get_nc():
    if "nc" not in _CACHED:
        _CACHED["nc"] = _build_nc()
    return _CACHED["nc"]


def kernel(q, k, v, w_q, w_k, w_v, w_fc1, w_fc2,
           ln1_g, ln1_b, ln2_g, ln2_b, ln3_g, ln3_b):
    import ml_dtypes

    q = np.asarray(q, dtype=np.float32)
    k = np.asarray(k, dtype=np.float32)
    v = np.asarray(v, dtype=np.float32)

    # fold LN gain into the projection weights; 1/sqrt(d_k) into w_q
    wq_s = (np.asarray(w_q) * np.asarray(ln1_g)[:, None] / np.sqrt(D_K)).astype(np.float32)
    wk_s = (np.asarray(w_k) * np.asarray(ln2_g)[:, None]).astype(np.float32)
    wv_s = (np.asarray(w_v) * np.asarray(ln3_g)[:, None]).astype(np.float32)
    for name, bb in (("ln1_b", ln1_b), ("ln2_b", ln2_b), ("ln3_b", ln3_b)):
        if np.any(np.asarray(bb)):
            raise NotImplementedError(f"nonzero {name} not supported")

    eyep = np.eye(128, dtype=np.float32)
    eyebp = eyep.astype(ml_dtypes.bfloat16)
    eyebn = (eyep * NEG_BIG).astype(ml_dtypes.bfloat16)

    nc = _get_nc()
    common = {
        "wq": np.ascontiguousarray(wq_s), "wk": np.ascontiguousarray(wk_s),
        "wv": np.ascontiguousarray(wv_s),
        "w1": np.ascontiguousarray(np.asarray(w_fc1, dtype=np.float32)),
        "w2": np.ascontiguousarray(np.asarray(w_fc2, dtype=np.float32)),
        "eyer": eyep, "eyebp": eyebp, "eyebn": eyebn,
        "onesr": np.ones((1, 128), dtype=np.float32),
        "onescol": np.ones((128, N_HEAD), dtype=np.float32),
    }
    in_maps = [
        {"xq": np.ascontiguousarray(q[b]), "xk": np.ascontiguousarray(k[b]),
         "xv": np.ascontiguousarray(v[b]), **common}
        for b in range(N_CORES)
    ]
    res = bass_utils.run_bass_kernel_spmd(
        nc, in_maps, core_ids=list(range(N_CORES)), **_CACHED.get("run_kwargs", {}))
    _CACHED["last_result"] = res

    dynamic = np.stack([res.results[b]["dyn"] for b in range(N_CORES)])
    static = np.stack([res.results[b]["stat"] for b in range(N_CORES)])
    at = np.stack([res.results[b]["attn_t"] for b in range(N_CORES)], axis=1)
    attn_flat = np.ascontiguousarray(at.transpose(0, 1, 3, 2)).reshape(
        N_HEAD * B, L, L)
    return dynamic, static, attn_flat


# revision 19
# speedup vs baseline: 1.0149x; 1.0149x over previous
"""Trainium2 Bass kernel for nn_MultiHeadAttention_45921790329378.

Full (unsharded) inputs in, full outputs back. Internally shards the
batch dimension across 8 NeuronCores (B=8 -> one batch per core, all 8
heads per core); weights replicated.

Per-core dataflow (single NeuronCore, Tile framework, fp32r matmuls):
  phase 1: layernorm q/k/v (bn_stats, batched rsqrt; LN gain folded into
           projection weights host-side), PE-transpose normalized
           activations to put d_in on partitions.
  phase 1b: projections (fp32r). qhT/khT/vhT in [d, l] head-pair tiles
           (head 2p rows 0-63, head 2p+1 rows 64-127), vh in [l, d]
           with an appended ones column for the softmax denominator.
  phase 2: per head-pair, per q-half: scoresT[k, q] via K=64 fp32r
           matmuls on disjoint PE row groups (concurrent); diagonal mask
           injected into the PSUM accumulation with a bf16 I.T @
           (-1e32*I) matmul; exp on ACT straight from PSUM; PV matmuls
           with lhsT=[vh|1] yield O^T and the denominator row Z; Zinv is
           partition-broadcast with a K=1 matmul, and the exp tiles are
           normalized elementwise (DVE + GPSIMD) and written out as
           attn^T.
  phase 3: dynamic = O @ w_fc1, static = vh_all @ w_fc2 from the [hd, l]
           operands produced above.

attn leaves the device as [h, k, q] per batch; the host transposes to
attn_flat[h*B+b, q, k] (pure layout rearrangement of device values).
"""

import numpy as np

import concourse.bacc as bacc
import concourse.mybir as mybir
import concourse.tile as tile
from concourse import bass_utils

N_CORES = 8
B, L, D_IN = 8, 1024, 512
N_HEAD, D_K, D_V, D_MODEL = 8, 64, 64, 512
LC = L // 128          # 8 l-chunks of 128
KC = D_IN // 128       # 4 d_in-chunks of 128
NEG_BIG = -1e32
EPS = 1e-5

F32 = mybir.dt.float32
F32R = mybir.dt.float32r
BF16 = mybir.dt.bfloat16
AF = mybir.ActivationFunctionType
ALU = mybir.AluOpType

# of the 8 per-pair attn-normalize multiplies, how many on gpsimd
ATTN_TT_ON_GPSIMD = 2

_CACHED = {}


def _build_nc():
    nc = bacc.Bacc("TRN2", target_bir_lowering=False, debug=False)

    xq = nc.dram_tensor("xq", [L, D_IN], F32, kind="ExternalInput")
    xk = nc.dram_tensor("xk", [L, D_IN], F32, kind="ExternalInput")
    xv = nc.dram_tensor("xv", [L, D_IN], F32, kind="ExternalInput")
    wq = nc.dram_tensor("wq", [D_IN, D_IN], F32R, kind="ExternalInput")
    wk = nc.dram_tensor("wk", [D_IN, D_IN], F32R, kind="ExternalInput")
    wv = nc.dram_tensor("wv", [D_IN, D_IN], F32R, kind="ExternalInput")
    w1 = nc.dram_tensor("w1", [D_IN, D_MODEL], F32R, kind="ExternalInput")
    w2 = nc.dram_tensor("w2", [D_IN, D_MODEL], F32R, kind="ExternalInput")
    eyer = nc.dram_tensor("eyer", [128, 128], F32R, kind="ExternalInput")
    eyebp = nc.dram_tensor("eyebp", [128, 128], BF16, kind="ExternalInput")
    eyebn = nc.dram_tensor("eyebn", [128, 128], BF16, kind="ExternalInput")
    onesr = nc.dram_tensor("onesr", [1, 128], F32, kind="ExternalInput")
    onescol = nc.dram_tensor("onescol", [128, N_HEAD], F32R, kind="ExternalInput")

    attn_t = nc.dram_tensor("attn_t", [N_HEAD, L, L], F32, kind="ExternalOutput")
    dyn = nc.dram_tensor("dyn", [L, D_MODEL], F32, kind="ExternalOutput")
    stat = nc.dram_tensor("stat", [L, D_MODEL], F32, kind="ExternalOutput")

    with tile.TileContext(nc) as tc:
        _emit(nc, tc, xq, xk, xv, wq, wk, wv, w1, w2, eyer, eyebp, eyebn,
              onesr, onescol, attn_t, dyn, stat)
    nc.compile()
    return nc


def _emit(nc, tc, xq, xk, xv, wq, wk, wv, w1, w2, eyer, eyebp, eyebn,
          onesr, onescol, attn_t, dyn, stat):
    from contextlib import ExitStack
    ctx = ExitStack()
    with ctx:
        consts = ctx.enter_context(tc.tile_pool(name="consts", bufs=1))
        w1_sb = [consts.tile([128, D_MODEL], F32R, name=f"w1_{kc}") for kc in range(KC)]
        w2_sb = [consts.tile([128, D_MODEL], F32R, name=f"w2_{kc}") for kc in range(KC)]
        for kc in range(KC):
            nc.sync.dma_start(out=w1_sb[kc], in_=w1.ap()[kc * 128:(kc + 1) * 128, :])
            nc.sync.dma_start(out=w2_sb[kc], in_=w2.ap()[kc * 128:(kc + 1) * 128, :])
        eyer_sb = consts.tile([128, 128], F32R, name="eyer_sb")
        eyebp_sb = consts.tile([128, 128], BF16, name="eyebp_sb")
        eyebn_sb = consts.tile([128, 128], BF16, name="eyebn_sb")
        nc.sync.dma_start(out=eyer_sb, in_=eyer.ap())
        nc.sync.dma_start(out=eyebp_sb, in_=eyebp.ap())
        nc.sync.dma_start(out=eyebn_sb, in_=eyebn.ap())
        ones1 = consts.tile([1, 128], F32, name="ones1")
        nc.sync.dma_start(out=ones1, in_=onesr.ap())
        eps_sb = consts.tile([128, 1], F32, name="eps_sb")
        nc.vector.memset(eps_sb, EPS)

        # persistent activation layouts (fp32r)
        persist = ctx.enter_context(tc.tile_pool(name="persist", bufs=1))
        qhT = [persist.tile([128, L], F32R, name=f"qhT_{p}") for p in range(4)]
        khT = [persist.tile([128, L], F32R, name=f"khT_{p}") for p in range(4)]
        vhT = [persist.tile([128, L], F32R, name=f"vhT_{p}") for p in range(4)]
        vhp = [persist.tile([128, N_HEAD, D_V + 1], F32R, name=f"vhp_{c}")
               for c in range(LC)]
        otp = [persist.tile([128, L], F32R, name=f"otp_{p}") for p in range(4)]

        # ---------------- phase 1: LN + transpose + projections ----------
        with tc.tile_pool(name="p1_w", bufs=1) as p1_w, \
             tc.tile_pool(name="p1_xnt", bufs=1) as p1_xnt, \
             tc.tile_pool(name="p1_x", bufs=9) as p1_x, \
             tc.tile_pool(name="p1_sb", bufs=3) as p1_sb, \
             tc.tile_pool(name="p1_st", bufs=2) as p1_st, \
             tc.tile_pool(name="p1_ps_tr", bufs=4, space="PSUM") as p1_ps_tr, \
             tc.tile_pool(name="p1_ps_pj", bufs=3, space="PSUM") as p1_ps_pj:

            wq_sb = [p1_w.tile([128, D_IN], F32R, name=f"wq_{kc}") for kc in range(KC)]
            wk_sb = [p1_w.tile([128, D_IN], F32R, name=f"wk_{kc}") for kc in range(KC)]
            wv_sb = [p1_w.tile([128, D_IN], F32R, name=f"wv_{kc}") for kc in range(KC)]
            for kc in range(KC):
                nc.sync.dma_start(out=wq_sb[kc], in_=wq.ap()[kc * 128:(kc + 1) * 128, :])
                nc.sync.dma_start(out=wk_sb[kc], in_=wk.ap()[kc * 128:(kc + 1) * 128, :])
                nc.sync.dma_start(out=wv_sb[kc], in_=wv.ap()[kc * 128:(kc + 1) * 128, :])

            xnT = {}
            for t in range(3):
                for kc in range(KC):
                    xnT[(t, kc)] = p1_xnt.tile([128, L], F32R, name=f"xnT_{t}_{kc}")

            for t, x_in in enumerate([xq, xk, xv]):
                xts = []
                mvall = p1_st.tile([128, 2 * LC], F32, name="mvall", tag="mvall")
                for c in range(LC):
                    xt = p1_x.tile([128, D_IN], F32, name="xt", tag="xt")
                    nc.sync.dma_start(out=xt, in_=x_in.ap()[c * 128:(c + 1) * 128, :])
                    xts.append(xt)
                    stt = p1_sb.tile([128, 6], F32, name="stt", tag="stt")
                    nc.vector.bn_stats(out=stt, in_=xt)
                    nc.vector.bn_aggr(out=mvall[:, 2 * c:2 * c + 2], in_=stt)
                # batched rstd: sig = sqrt(var + eps); rs = 1/sig
                sg = p1_st.tile([128, LC], F32, name="sg", tag="sg")
                var_view = mvall.rearrange("p (c two) -> p c two", two=2)[:, :, 1]
                nc.scalar.activation(sg, var_view, AF.Sqrt, bias=eps_sb)
                rs = p1_st.tile([128, LC], F32, name="rs", tag="rs")
                nc.vector.reciprocal(out=rs, in_=sg)
                ps_tr = {}
                for c in range(LC):
                    xn = p1_sb.tile([128, D_IN], F32R, name="xn", tag="xn")
                    nc.vector.tensor_scalar(
                        out=xn, in0=xts[c], scalar1=mvall[:, 2 * c:2 * c + 1],
                        scalar2=rs[:, c:c + 1],
                        op0=ALU.subtract, op1=ALU.mult)
                    half = c // 4
                    for dc in range(KC):
                        key = (dc, half)
                        if key not in ps_tr:
                            ps_tr[key] = p1_ps_tr.tile([128, 512], F32R,
                                                       name="ps_tr", tag="ps_tr")
                        cc = c % 4
                        nc.tensor.transpose(
                            ps_tr[key][:, cc * 128:(cc + 1) * 128],
                            xn[:, dc * 128:(dc + 1) * 128], eyer_sb)
                        if cc == 3:
                            nc.scalar.copy(
                                out=xnT[(t, dc)][:, half * 512:(half + 1) * 512],
                                in_=ps_tr[key])
                            del ps_tr[key]

            # projections: qhT/khT/vhT head-pair [d, l] tiles (fp32r)
            for src, (wt, dstT) in enumerate(
                    ((wq_sb, qhT), (wk_sb, khT), (wv_sb, vhT))):
                for p in range(4):
                    for lh in range(2):
                        pj = p1_ps_pj.tile([128, 512], F32, name="pj", tag="pj")
                        for kc in range(KC):
                            nc.tensor.matmul(
                                pj, wt[kc][:, p * 128:(p + 1) * 128],
                                xnT[(src, kc)][:, lh * 512:(lh + 1) * 512],
                                start=(kc == 0), stop=(kc == KC - 1))
                        nc.scalar.copy(out=dstT[p][:, lh * 512:(lh + 1) * 512],
                                       in_=pj)
            # vh natural [l, d] with ones column
            for c in range(LC):
                pj = p1_ps_pj.tile([128, 512], F32, name="pjv", tag="pj")
                for kc in range(KC):
                    nc.tensor.matmul(
                        pj, xnT[(2, kc)][:, c * 128:(c + 1) * 128], wv_sb[kc],
                        start=(kc == 0), stop=(kc == KC - 1))
                nc.vector.tensor_copy(
                    out=vhp[c][:, :, 0:D_V],
                    in_=pj.rearrange("p (h d) -> p h d", h=N_HEAD))
                nc.sync.dma_start(out=vhp[c][:, :, D_V], in_=onescol.ap())

        # ---------------- phase 2: attention per head-pair ---------------
        # et tiles are [128 k, 2048] = (qh 2, head-in-pair 2, q' 512)
        # zb tiles share that layout; attn normalization is one in-place
        # [128, 2048] multiply per (pair, k-chunk), then a 1 MB DMA.
        with tc.tile_pool(name="p2_et", bufs=8) as p2_et, \
             tc.tile_pool(name="p2_z", bufs=2) as p2_z, \
             tc.tile_pool(name="p2_zb", bufs=1) as p2_zb, \
             tc.tile_pool(name="ps_s", bufs=2, space="PSUM") as ps_s_pool, \
             tc.tile_pool(name="ps_ot", bufs=1, space="PSUM") as ps_ot_pool, \
             tc.tile_pool(name="ps_zb", bufs=1, space="PSUM") as ps_zb_pool:

            for pr in range(4):
                hA = 2 * pr
                ets = {}
                zb_sb = p2_zb.tile([128, 2 * L], F32, name="zb_sb", tag="zb")
                for qh in range(2):
                    qs = slice(qh * 512, (qh + 1) * 512)
                    ps_ot = ps_ot_pool.tile([D_V + 1, L], F32, name="ps_ot",
                                            tag="ps_ot")
                    for kc in range(LC):
                        ps_s = ps_s_pool.tile([128, L], F32, name="ps_s",
                                              tag="ps_s")
                        diag_here = (qh * 512 <= kc * 128 < (qh + 1) * 512)
                        for hh in range(2):  # head in pair; PE row groups
                            o = 64 * hh
                            nc.tensor.matmul(
                                ps_s[:, hh * 512:(hh + 1) * 512],
                                khT[pr][o:o + 64, kc * 128:(kc + 1) * 128],
                                qhT[pr][o:o + 64, qs],
                                start=True, stop=not diag_here)
                        if diag_here:
                            d0 = kc * 128 - qh * 512
                            for hh in range(2):
                                nc.tensor.matmul(
                                    ps_s[:, hh * 512 + d0:hh * 512 + d0 + 128],
                                    eyebp_sb, eyebn_sb, start=False, stop=True)
                        if qh == 0:
                            ets[kc] = p2_et.tile([128, 2 * L], F32R,
                                                 name="et", tag="et")
                        et = ets[kc]
                        nc.scalar.activation(et[:, qh * L:(qh + 1) * L], ps_s,
                                             AF.Exp)
                        for hh in range(2):
                            nc.tensor.matmul(
                                ps_ot[:, hh * 512:(hh + 1) * 512],
                                vhp[kc][:, hA + hh, :],
                                et[:, qh * L + hh * 512:qh * L + (hh + 1) * 512],
                                start=(kc == 0), stop=(kc == LC - 1))
                    zrow = p2_z.tile([1, L], F32, name="zrow", tag="zrow")
                    nc.scalar.copy(out=zrow, in_=ps_ot[D_V:D_V + 1, :])
                    zscr = p2_z.tile([1, L], F32, name="zscr", tag="zscr")
                    zinv = p2_z.tile([1, L], F32, name="zinv", tag="zinv")
                    nc.vector.reciprocal_approx_accurate(
                        out=zinv, in_=zrow, scratch=zscr)
                    ps_zb = ps_zb_pool.tile([128, L], F32, name="ps_zb",
                                            tag="ps_zb")
                    for hh in range(2):
                        nc.tensor.matmul(ps_zb[:, hh * 512:(hh + 1) * 512],
                                         ones1, zinv[:, hh * 512:(hh + 1) * 512],
                                         start=True, stop=True)
                    nc.scalar.copy(out=zb_sb[:, qh * L:(qh + 1) * L], in_=ps_zb)
                    # O^T rows: head A -> otp rows 0-63, head B -> rows 64-127
                    for hh in range(2):
                        o = 64 * hh
                        nc.scalar.copy(out=otp[pr][o:o + 64, qs],
                                       in_=ps_ot[0:D_V, hh * 512:(hh + 1) * 512])
                        nc.vector.tensor_mul(
                            out=otp[pr][o:o + 64, qs],
                            in0=otp[pr][o:o + 64, qs],
                            in1=zb_sb[o:o + 64, qh * L + hh * 512:
                                      qh * L + (hh + 1) * 512])
                # normalized attn^T -> HBM (1 MB per DMA)
                for kc in range(LC):
                    et = ets[kc]
                    eng = nc.gpsimd if kc < ATTN_TT_ON_GPSIMD else nc.vector
                    with nc.allow_low_precision(reason="fp32r attn tiles"):
                        eng.tensor_mul(out=et, in0=et, in1=zb_sb)
                    dst = attn_t.ap()[hA:hA + 2, kc * 128:(kc + 1) * 128,
                                      :].rearrange("h k (a q) -> k a h q", a=2)
                    nc.sync.dma_start(
                        out=dst,
                        in_=et.bitcast(F32).rearrange("p (a h q) -> p a h q",
                                                      a=2, h=2))

        # ---------------- phase 3: dynamic / static ----------------------
        with tc.tile_pool(name="p3_sb", bufs=4) as p3_sb, \
             tc.tile_pool(name="p3_ps", bufs=4, space="PSUM") as p3_ps:
            for c in range(LC):
                pd = p3_ps.tile([128, D_MODEL], F32, name="pd", tag="pd")
                for p in range(4):
                    nc.tensor.matmul(pd, otp[p][:, c * 128:(c + 1) * 128],
                                     w1_sb[p], start=(p == 0), stop=(p == 3))
                db = p3_sb.tile([128, D_MODEL], F32, name="db", tag="db")
                nc.scalar.copy(out=db, in_=pd)
                nc.sync.dma_start(out=dyn.ap()[c * 128:(c + 1) * 128, :], in_=db)

                ps2 = p3_ps.tile([128, D_MODEL], F32, name="ps2", tag="pd")
                for p in range(4):
                    nc.tensor.matmul(ps2, vhT[p][:, c * 128:(c + 1) * 128],
                                     w2_sb[p], start=(p == 0), stop=(p == 3))
                sb2 = p3_sb.tile([128, D_MODEL], F32, name="sb2", tag="db")
                nc.scalar.copy(out=sb2, in_=ps2)
                nc.sync.dma_start(out=stat.ap()[c * 128:(c + 1) * 128, :], in_=sb2)


def _get_nc():
    if "nc" not in _CACHED:
        _CACHED["nc"] = _build_nc()
    return _CACHED["nc"]


def kernel(q, k, v, w_q, w_k, w_v, w_fc1, w_fc2,
           ln1_g, ln1_b, ln2_g, ln2_b, ln3_g, ln3_b):
    import ml_dtypes

    q = np.asarray(q, dtype=np.float32)
    k = np.asarray(k, dtype=np.float32)
    v = np.asarray(v, dtype=np.float32)

    # fold LN gain into the projection weights; 1/sqrt(d_k) into w_q
    wq_s = (np.asarray(w_q) * np.asarray(ln1_g)[:, None] / np.sqrt(D_K)).astype(np.float32)
    wk_s = (np.asarray(w_k) * np.asarray(ln2_g)[:, None]).astype(np.float32)
    wv_s = (np.asarray(w_v) * np.asarray(ln3_g)[:, None]).astype(np.float32)
    for name, bb in (("ln1_b", ln1_b), ("ln2_b", ln2_b), ("ln3_b", ln3_b)):
        if np.any(np.asarray(bb)):
            raise NotImplementedError(f"nonzero {name} not supported")

    eyep = np.eye(128, dtype=np.float32)
    eyebp = eyep.astype(ml_dtypes.bfloat16)
    eyebn = (eyep * NEG_BIG).astype(ml_dtypes.bfloat16)

    nc = _get_nc()
    common = {
        "wq": np.ascontiguousarray(wq_s), "wk": np.ascontiguousarray(wk_s),
        "wv": np.ascontiguousarray(wv_s),
        "w1": np.ascontiguousarray(np.asarray(w_fc1, dtype=np.float32)),
        "w2": np.ascontiguousarray(np.asarray(w_fc2, dtype=np.float32)),
        "eyer": eyep, "eyebp": eyebp, "eyebn": eyebn,
        "onesr": np.ones((1, 128), dtype=np.float32),
        "onescol": np.ones((128, N_HEAD), dtype=np.float32),
    }
    in_maps = [
        {"xq": np.ascontiguousarray(q[b]), "xk": np.ascontiguousarray(k[b]),
         "xv": np.ascontiguousarray(v[b]), **common}
        for b in range(N_CORES)
    ]
    res = bass_utils.run_bass_kernel_spmd(
        nc, in_maps, core_ids=list(range(N_CORES)), **_CACHED.get("run_kwargs", {}))
    _CACHED["last_result"] = res

    dynamic = np.stack([res.results[b]["dyn"] for b in range(N_CORES)])
    static = np.stack([res.results[b]["stat"] for b in range(N_CORES)])
    at = np.stack([res.results[b]["attn_t"] for b in range(N_CORES)], axis=1)
    attn_flat = np.ascontiguousarray(at.transpose(0, 1, 3, 2)).reshape(
        N_HEAD * B, L, L)
    return dynamic, static, attn_flat


# revision 20
# speedup vs baseline: 1.1828x; 1.1654x over previous
"""Trainium2 Bass kernel for nn_MultiHeadAttention_45921790329378.

Full (unsharded) inputs in, full outputs back. Internally shards the
batch dimension across 8 NeuronCores (B=8 -> one batch per core, all 8
heads per core); weights replicated.

Per-core dataflow (single NeuronCore, Tile framework, fp32r matmuls):
  phase 1: layernorm q/k/v (bn_stats, batched rsqrt; LN gain folded into
           projection weights host-side), PE-transpose normalized
           activations to put d_in on partitions.
  phase 1b: projections (fp32r). qhT/khT/vhT in [d, l] head-pair tiles
           (head 2p rows 0-63, head 2p+1 rows 64-127), vh in [l, d]
           with an appended ones column for the softmax denominator.
  phase 2: per head-pair, per q-half: scoresT[k, q] via K=64 fp32r
           matmuls on disjoint PE row groups (concurrent); diagonal mask
           injected into the PSUM accumulation with a bf16 I.T @
           (-1e32*I) matmul; exp on ACT straight from PSUM; PV matmuls
           with lhsT=[vh|1] yield O^T and the denominator row Z; Zinv is
           partition-broadcast with a K=1 matmul, and the exp tiles are
           normalized elementwise (DVE + GPSIMD) and written out as
           attn^T.
  phase 3: dynamic = O @ w_fc1, static = vh_all @ w_fc2 from the [hd, l]
           operands produced above.

attn leaves the device as [h, k, q] per batch; the host transposes to
attn_flat[h*B+b, q, k] (pure layout rearrangement of device values).
"""

import numpy as np

import concourse.bacc as bacc
import concourse.mybir as mybir
import concourse.tile as tile
from concourse import bass_utils

N_CORES = 8
B, L, D_IN = 8, 1024, 512
N_HEAD, D_K, D_V, D_MODEL = 8, 64, 64, 512
LC = L // 128          # 8 l-chunks of 128
KC = D_IN // 128       # 4 d_in-chunks of 128
NEG_BIG = -1e32
EPS = 1e-5

F32 = mybir.dt.float32
F32R = mybir.dt.float32r
BF16 = mybir.dt.bfloat16
AF = mybir.ActivationFunctionType
ALU = mybir.AluOpType

# of the 8 per-pair attn-normalize multiplies, how many on gpsimd
ATTN_TT_ON_GPSIMD = 2

_CACHED = {}


def _build_nc():
    nc = bacc.Bacc("TRN2", target_bir_lowering=False, debug=False)

    xq = nc.dram_tensor("xq", [L, D_IN], F32, kind="ExternalInput")
    xk = nc.dram_tensor("xk", [L, D_IN], F32, kind="ExternalInput")
    xv = nc.dram_tensor("xv", [L, D_IN], F32, kind="ExternalInput")
    wq = nc.dram_tensor("wq", [D_IN, D_IN], F32R, kind="ExternalInput")
    wk = nc.dram_tensor("wk", [D_IN, D_IN], F32R, kind="ExternalInput")
    wv = nc.dram_tensor("wv", [D_IN, D_IN], F32R, kind="ExternalInput")
    w1 = nc.dram_tensor("w1", [D_IN, D_MODEL], F32R, kind="ExternalInput")
    w2 = nc.dram_tensor("w2", [D_IN, D_MODEL], F32R, kind="ExternalInput")
    eyer = nc.dram_tensor("eyer", [128, 128], F32R, kind="ExternalInput")
    eyebp = nc.dram_tensor("eyebp", [128, 128], BF16, kind="ExternalInput")
    eyebn = nc.dram_tensor("eyebn", [128, 128], BF16, kind="ExternalInput")
    onesr = nc.dram_tensor("onesr", [1, 128], F32, kind="ExternalInput")
    onescol = nc.dram_tensor("onescol", [128, N_HEAD], F32R, kind="ExternalInput")

    attn_t = nc.dram_tensor("attn_t", [N_HEAD, L, L], F32, kind="ExternalOutput")
    dyn = nc.dram_tensor("dyn", [L, D_MODEL], F32, kind="ExternalOutput")
    stat = nc.dram_tensor("stat", [L, D_MODEL], F32, kind="ExternalOutput")

    with tile.TileContext(nc) as tc:
        _emit(nc, tc, xq, xk, xv, wq, wk, wv, w1, w2, eyer, eyebp, eyebn,
              onesr, onescol, attn_t, dyn, stat)
    nc.compile()
    return nc


def _emit(nc, tc, xq, xk, xv, wq, wk, wv, w1, w2, eyer, eyebp, eyebn,
          onesr, onescol, attn_t, dyn, stat):
    from contextlib import ExitStack
    ctx = ExitStack()
    with ctx:
        consts = ctx.enter_context(tc.tile_pool(name="consts", bufs=1))
        w1_sb = [consts.tile([128, D_MODEL], F32R, name=f"w1_{kc}") for kc in range(KC)]
        w2_sb = [consts.tile([128, D_MODEL], F32R, name=f"w2_{kc}") for kc in range(KC)]
        for kc in range(KC):
            nc.sync.dma_start(out=w1_sb[kc], in_=w1.ap()[kc * 128:(kc + 1) * 128, :])
            nc.sync.dma_start(out=w2_sb[kc], in_=w2.ap()[kc * 128:(kc + 1) * 128, :])
        eyer_sb = consts.tile([128, 128], F32R, name="eyer_sb")
        eyebp_sb = consts.tile([128, 128], BF16, name="eyebp_sb")
        eyebn_sb = consts.tile([128, 128], BF16, name="eyebn_sb")
        nc.sync.dma_start(out=eyer_sb, in_=eyer.ap())
        nc.sync.dma_start(out=eyebp_sb, in_=eyebp.ap())
        nc.sync.dma_start(out=eyebn_sb, in_=eyebn.ap())
        ones1 = consts.tile([1, 128], F32, name="ones1")
        nc.sync.dma_start(out=ones1, in_=onesr.ap())
        eps_sb = consts.tile([128, 1], F32, name="eps_sb")
        nc.vector.memset(eps_sb, EPS)

        # persistent activation layouts (fp32r)
        persist = ctx.enter_context(tc.tile_pool(name="persist", bufs=1))
        qhT = [persist.tile([128, L], F32R, name=f"qhT_{p}") for p in range(4)]
        khT = [persist.tile([128, L], F32R, name=f"khT_{p}") for p in range(4)]
        vhT = [persist.tile([128, L], F32R, name=f"vhT_{p}") for p in range(4)]
        vhp = [persist.tile([128, N_HEAD, D_V + 1], F32R, name=f"vhp_{c}")
               for c in range(LC)]
        otp = [persist.tile([128, L], F32R, name=f"otp_{p}") for p in range(4)]

        # ---------------- phase 1: LN + transpose + projections ----------
        with tc.tile_pool(name="p1_w", bufs=1) as p1_w, \
             tc.tile_pool(name="p1_xnt", bufs=1) as p1_xnt, \
             tc.tile_pool(name="p1_x", bufs=9) as p1_x, \
             tc.tile_pool(name="p1_sb", bufs=3) as p1_sb, \
             tc.tile_pool(name="p1_st", bufs=2) as p1_st, \
             tc.tile_pool(name="p1_ps_tr", bufs=4, space="PSUM") as p1_ps_tr, \
             tc.tile_pool(name="p1_ps_pj", bufs=3, space="PSUM") as p1_ps_pj:

            wq_sb = [p1_w.tile([128, D_IN], F32R, name=f"wq_{kc}") for kc in range(KC)]
            wk_sb = [p1_w.tile([128, D_IN], F32R, name=f"wk_{kc}") for kc in range(KC)]
            wv_sb = [p1_w.tile([128, D_IN], F32R, name=f"wv_{kc}") for kc in range(KC)]
            for kc in range(KC):
                nc.sync.dma_start(out=wq_sb[kc], in_=wq.ap()[kc * 128:(kc + 1) * 128, :])
                nc.sync.dma_start(out=wk_sb[kc], in_=wk.ap()[kc * 128:(kc + 1) * 128, :])
                nc.sync.dma_start(out=wv_sb[kc], in_=wv.ap()[kc * 128:(kc + 1) * 128, :])

            xnT = {}
            for t in range(3):
                for kc in range(KC):
                    xnT[(t, kc)] = p1_xnt.tile([128, L], F32R, name=f"xnT_{t}_{kc}")

            for t, x_in in enumerate([xq, xk, xv]):
                xts = []
                mvall = p1_st.tile([128, 2 * LC], F32, name="mvall", tag="mvall")
                for c in range(LC):
                    xt = p1_x.tile([128, D_IN], F32, name="xt", tag="xt")
                    nc.sync.dma_start(out=xt, in_=x_in.ap()[c * 128:(c + 1) * 128, :])
                    xts.append(xt)
                    stt = p1_sb.tile([128, 6], F32, name="stt", tag="stt")
                    nc.vector.bn_stats(out=stt, in_=xt)
                    nc.vector.bn_aggr(out=mvall[:, 2 * c:2 * c + 2], in_=stt)
                # batched rstd: sig = sqrt(var + eps); rs = 1/sig
                sg = p1_st.tile([128, LC], F32, name="sg", tag="sg")
                var_view = mvall.rearrange("p (c two) -> p c two", two=2)[:, :, 1]
                nc.scalar.activation(sg, var_view, AF.Sqrt, bias=eps_sb)
                rs = p1_st.tile([128, LC], F32, name="rs", tag="rs")
                nc.vector.reciprocal(out=rs, in_=sg)
                ps_tr = {}
                for c in range(LC):
                    xn = p1_sb.tile([128, D_IN], F32R, name="xn", tag="xn")
                    nc.vector.tensor_scalar(
                        out=xn, in0=xts[c], scalar1=mvall[:, 2 * c:2 * c + 1],
                        scalar2=rs[:, c:c + 1],
                        op0=ALU.subtract, op1=ALU.mult)
                    half = c // 4
                    for dc in range(KC):
                        key = (dc, half)
                        if key not in ps_tr:
                            ps_tr[key] = p1_ps_tr.tile([128, 512], F32R,
                                                       name="ps_tr", tag="ps_tr")
                        cc = c % 4
                        nc.tensor.transpose(
                            ps_tr[key][:, cc * 128:(cc + 1) * 128],
                            xn[:, dc * 128:(dc + 1) * 128], eyer_sb)
                        if cc == 3:
                            nc.scalar.copy(
                                out=xnT[(t, dc)][:, half * 512:(half + 1) * 512],
                                in_=ps_tr[key])
                            del ps_tr[key]

            # projections: qhT/khT/vhT head-pair [d, l] tiles (fp32r)
            for src, (wt, dstT) in enumerate(
                    ((wq_sb, qhT), (wk_sb, khT), (wv_sb, vhT))):
                for p in range(4):
                    for lh in range(2):
                        pj = p1_ps_pj.tile([128, 512], F32, name="pj", tag="pj")
                        for kc in range(KC):
                            nc.tensor.matmul(
                                pj, wt[kc][:, p * 128:(p + 1) * 128],
                                xnT[(src, kc)][:, lh * 512:(lh + 1) * 512],
                                start=(kc == 0), stop=(kc == KC - 1))
                        nc.scalar.copy(out=dstT[p][:, lh * 512:(lh + 1) * 512],
                                       in_=pj)
            # vh natural [l, d] with ones column
            for c in range(LC):
                pj = p1_ps_pj.tile([128, 512], F32, name="pjv", tag="pj")
                for kc in range(KC):
                    nc.tensor.matmul(
                        pj, xnT[(2, kc)][:, c * 128:(c + 1) * 128], wv_sb[kc],
                        start=(kc == 0), stop=(kc == KC - 1))
                nc.vector.tensor_copy(
                    out=vhp[c][:, :, 0:D_V],
                    in_=pj.rearrange("p (h d) -> p h d", h=N_HEAD))
                nc.sync.dma_start(out=vhp[c][:, :, D_V], in_=onescol.ap())

        # ---------------- phase 2: attention per head-pair ---------------
        # et tiles are [128 k, 1024] = (head-in-pair 2, q' 512) for one
        # q-half; normalization is per (pair, q-half, k-chunk): DVE tiles
        # read Zinv-broadcast from PSUM, gpsimd tiles from an SBUF copy.
        with tc.tile_pool(name="p2_et", bufs=13) as p2_et, \
             tc.tile_pool(name="p2_z", bufs=1) as p2_z, \
             tc.tile_pool(name="p2_zb", bufs=2) as p2_zb, \
             tc.tile_pool(name="ps_s", bufs=2, space="PSUM") as ps_s_pool, \
             tc.tile_pool(name="ps_ot", bufs=1, space="PSUM") as ps_ot_pool, \
             tc.tile_pool(name="ps_zb", bufs=1, space="PSUM") as ps_zb_pool:

            for pr in range(4):
                hA = 2 * pr
                for qh in range(2):
                    qs = slice(qh * 512, (qh + 1) * 512)
                    ps_ot = ps_ot_pool.tile([D_V + 1, L], F32, name="ps_ot",
                                            tag="ps_ot")
                    ets = []
                    for kc in range(LC):
                        ps_s = ps_s_pool.tile([128, L], F32, name="ps_s",
                                              tag="ps_s")
                        diag_here = (qh * 512 <= kc * 128 < (qh + 1) * 512)
                        for hh in range(2):  # head in pair; PE row groups
                            o = 64 * hh
                            nc.tensor.matmul(
                                ps_s[:, hh * 512:(hh + 1) * 512],
                                khT[pr][o:o + 64, kc * 128:(kc + 1) * 128],
                                qhT[pr][o:o + 64, qs],
                                start=True, stop=not diag_here)
                        if diag_here:
                            d0 = kc * 128 - qh * 512
                            for hh in range(2):
                                nc.tensor.matmul(
                                    ps_s[:, hh * 512 + d0:hh * 512 + d0 + 128],
                                    eyebp_sb, eyebn_sb, start=False, stop=True)
                        et = p2_et.tile([128, L], F32R, name="et", tag="et")
                        nc.scalar.activation(et, ps_s, AF.Exp)
                        ets.append(et)
                        for hh in range(2):
                            nc.tensor.matmul(
                                ps_ot[:, hh * 512:(hh + 1) * 512],
                                vhp[kc][:, hA + hh, :],
                                et[:, hh * 512:(hh + 1) * 512],
                                start=(kc == 0), stop=(kc == LC - 1))
                    zrow = p2_z.tile([1, L], F32, name="zrow", tag="zrow")
                    nc.scalar.copy(out=zrow, in_=ps_ot[D_V:D_V + 1, :])
                    zinv = p2_z.tile([1, L], F32, name="zinv", tag="zinv")
                    nc.vector.reciprocal_approx_fast(out=zinv, in_=zrow)
                    ps_zb = ps_zb_pool.tile([128, L], F32, name="ps_zb",
                                            tag="ps_zb")
                    for hh in range(2):
                        nc.tensor.matmul(ps_zb[:, hh * 512:(hh + 1) * 512],
                                         ones1, zinv[:, hh * 512:(hh + 1) * 512],
                                         start=True, stop=True)
                    zb_sb = p2_zb.tile([128, L], F32, name="zb_sb", tag="zb")
                    nc.scalar.copy(out=zb_sb, in_=ps_zb)
                    # O^T rows: head A -> otp rows 0-63, head B -> rows 64-127
                    for hh in range(2):
                        o = 64 * hh
                        nc.scalar.copy(out=otp[pr][o:o + 64, qs],
                                       in_=ps_ot[0:D_V, hh * 512:(hh + 1) * 512])
                        nc.vector.tensor_mul(
                            out=otp[pr][o:o + 64, qs],
                            in0=otp[pr][o:o + 64, qs],
                            in1=zb_sb[o:o + 64, hh * 512:(hh + 1) * 512])
                    # normalized attn^T -> HBM (512 KB per DMA)
                    for kc in range(LC):
                        et = ets[kc]
                        eng = nc.gpsimd if kc < ATTN_TT_ON_GPSIMD else nc.vector
                        zsrc = zb_sb if kc < ATTN_TT_ON_GPSIMD else ps_zb
                        with nc.allow_low_precision(reason="fp32r attn tiles"):
                            eng.tensor_mul(out=et, in0=et, in1=zsrc)
                        dst = attn_t.ap()[hA:hA + 2, kc * 128:(kc + 1) * 128,
                                          qs].rearrange("h k q -> k h q")
                        nc.sync.dma_start(
                            out=dst,
                            in_=et.bitcast(F32).rearrange("p (h q) -> p h q",
                                                          h=2))

        # ---------------- phase 3: dynamic / static ----------------------
        with tc.tile_pool(name="p3_sb", bufs=2) as p3_sb, \
             tc.tile_pool(name="p3_ps", bufs=4, space="PSUM") as p3_ps:
            for c in range(LC):
                pd = p3_ps.tile([128, D_MODEL], F32, name="pd", tag="pd")
                for p in range(4):
                    nc.tensor.matmul(pd, otp[p][:, c * 128:(c + 1) * 128],
                                     w1_sb[p], start=(p == 0), stop=(p == 3))
                db = p3_sb.tile([128, D_MODEL], F32, name="db", tag="db")
                nc.scalar.copy(out=db, in_=pd)
                nc.sync.dma_start(out=dyn.ap()[c * 128:(c + 1) * 128, :], in_=db)

                ps2 = p3_ps.tile([128, D_MODEL], F32, name="ps2", tag="pd")
                for p in range(4):
                    nc.tensor.matmul(ps2, vhT[p][:, c * 128:(c + 1) * 128],
                                     w2_sb[p], start=(p == 0), stop=(p == 3))
                sb2 = p3_sb.tile([128, D_MODEL], F32, name="sb2", tag="db")
                nc.scalar.copy(out=sb2, in_=ps2)
                nc.sync.dma_start(out=stat.ap()[c * 128:(c + 1) * 128, :], in_=sb2)


def _get_nc():
    if "nc" not in _CACHED:
        _CACHED["nc"] = _build_nc()
    return _CACHED["nc"]


def kernel(q, k, v, w_q, w_k, w_v, w_fc1, w_fc2,
           ln1_g, ln1_b, ln2_g, ln2_b, ln3_g, ln3_b):
    import ml_dtypes

    q = np.asarray(q, dtype=np.float32)
    k = np.asarray(k, dtype=np.float32)
    v = np.asarray(v, dtype=np.float32)

    # fold LN gain into the projection weights; 1/sqrt(d_k) into w_q
    wq_s = (np.asarray(w_q) * np.asarray(ln1_g)[:, None] / np.sqrt(D_K)).astype(np.float32)
    wk_s = (np.asarray(w_k) * np.asarray(ln2_g)[:, None]).astype(np.float32)
    wv_s = (np.asarray(w_v) * np.asarray(ln3_g)[:, None]).astype(np.float32)
    for name, bb in (("ln1_b", ln1_b), ("ln2_b", ln2_b), ("ln3_b", ln3_b)):
        if np.any(np.asarray(bb)):
            raise NotImplementedError(f"nonzero {name} not supported")

    eyep = np.eye(128, dtype=np.float32)
    eyebp = eyep.astype(ml_dtypes.bfloat16)
    eyebn = (eyep * NEG_BIG).astype(ml_dtypes.bfloat16)

    nc = _get_nc()
    common = {
        "wq": np.ascontiguousarray(wq_s), "wk": np.ascontiguousarray(wk_s),
        "wv": np.ascontiguousarray(wv_s),
        "w1": np.ascontiguousarray(np.asarray(w_fc1, dtype=np.float32)),
        "w2": np.ascontiguousarray(np.asarray(w_fc2, dtype=np.float32)),
        "eyer": eyep, "eyebp": eyebp, "eyebn": eyebn,
        "onesr": np.ones((1, 128), dtype=np.float32),
        "onescol": np.ones((128, N_HEAD), dtype=np.float32),
    }
    in_maps = [
        {"xq": np.ascontiguousarray(q[b]), "xk": np.ascontiguousarray(k[b]),
         "xv": np.ascontiguousarray(v[b]), **common}
        for b in range(N_CORES)
    ]
    res = bass_utils.run_bass_kernel_spmd(
        nc, in_maps, core_ids=list(range(N_CORES)), **_CACHED.get("run_kwargs", {}))
    _CACHED["last_result"] = res

    dynamic = np.stack([res.results[b]["dyn"] for b in range(N_CORES)])
    static = np.stack([res.results[b]["stat"] for b in range(N_CORES)])
    at = np.stack([res.results[b]["attn_t"] for b in range(N_CORES)], axis=1)
    attn_flat = np.ascontiguousarray(at.transpose(0, 1, 3, 2)).reshape(
        N_HEAD * B, L, L)
    return dynamic, static, attn_flat


# revision 21
# speedup vs baseline: 1.2558x; 1.0617x over previous
"""Trainium2 Bass kernel for nn_MultiHeadAttention_45921790329378.

Full (unsharded) inputs in, full outputs back. Internally shards the
batch dimension across 8 NeuronCores (B=8 -> one batch per core, all 8
heads per core); weights replicated.

Per-core dataflow (single NeuronCore, Tile framework, fp32r matmuls):
  phase 1: layernorm q/k/v (bn_stats, batched rsqrt; LN gain folded into
           projection weights host-side), PE-transpose normalized
           activations to put d_in on partitions.
  phase 1b: projections (fp32r). qhT/khT/vhT in [d, l] head-pair tiles
           (head 2p rows 0-63, head 2p+1 rows 64-127), vh in [l, d]
           with an appended ones column for the softmax denominator.
  phase 2: per head-pair, per q-half: scoresT[k, q] via K=64 fp32r
           matmuls on disjoint PE row groups (concurrent); diagonal mask
           injected into the PSUM accumulation with a bf16 I.T @
           (-1e32*I) matmul; exp on ACT straight from PSUM; PV matmuls
           with lhsT=[vh|1] yield O^T and the denominator row Z; Zinv is
           partition-broadcast with a K=1 matmul, and the exp tiles are
           normalized elementwise (DVE + GPSIMD) and written out as
           attn^T.
  phase 3: dynamic = O @ w_fc1, static = vh_all @ w_fc2 from the [hd, l]
           operands produced above.

attn leaves the device as [h, k, q] per batch; the host transposes to
attn_flat[h*B+b, q, k] (pure layout rearrangement of device values).
"""

import numpy as np

import concourse.bacc as bacc
import concourse.mybir as mybir
import concourse.tile as tile
from concourse import bass_utils

N_CORES = 8
B, L, D_IN = 8, 1024, 512
N_HEAD, D_K, D_V, D_MODEL = 8, 64, 64, 512
LC = L // 128          # 8 l-chunks of 128
KC = D_IN // 128       # 4 d_in-chunks of 128
NEG_BIG = -1e32
EPS = 1e-5

F32 = mybir.dt.float32
F32R = mybir.dt.float32r
BF16 = mybir.dt.bfloat16
AF = mybir.ActivationFunctionType
ALU = mybir.AluOpType

# of the 8 per-pair attn-normalize multiplies, how many on gpsimd
ATTN_TT_ON_GPSIMD = 2

_CACHED = {}


def _build_nc():
    nc = bacc.Bacc("TRN2", target_bir_lowering=False, debug=False)

    xq = nc.dram_tensor("xq", [L, D_IN], F32, kind="ExternalInput")
    xk = nc.dram_tensor("xk", [L, D_IN], F32, kind="ExternalInput")
    xv = nc.dram_tensor("xv", [L, D_IN], F32, kind="ExternalInput")
    wq = nc.dram_tensor("wq", [D_IN, D_IN], F32R, kind="ExternalInput")
    wk = nc.dram_tensor("wk", [D_IN, D_IN], F32R, kind="ExternalInput")
    wv = nc.dram_tensor("wv", [D_IN, D_IN], F32R, kind="ExternalInput")
    w1 = nc.dram_tensor("w1", [D_IN, D_MODEL], F32R, kind="ExternalInput")
    w2 = nc.dram_tensor("w2", [D_IN, D_MODEL], F32R, kind="ExternalInput")
    eyer = nc.dram_tensor("eyer", [128, 128], F32R, kind="ExternalInput")
    eyebp = nc.dram_tensor("eyebp", [128, 128], BF16, kind="ExternalInput")
    eyebn = nc.dram_tensor("eyebn", [128, 128], BF16, kind="ExternalInput")
    onesr = nc.dram_tensor("onesr", [1, 128], F32, kind="ExternalInput")
    onescol = nc.dram_tensor("onescol", [128, N_HEAD], F32R, kind="ExternalInput")

    attn_t = nc.dram_tensor("attn_t", [4, 2, LC, 128, 2, 512], F32,
                            kind="ExternalOutput")
    dyn = nc.dram_tensor("dyn", [L, D_MODEL], F32, kind="ExternalOutput")
    stat = nc.dram_tensor("stat", [L, D_MODEL], F32, kind="ExternalOutput")

    with tile.TileContext(nc) as tc:
        _emit(nc, tc, xq, xk, xv, wq, wk, wv, w1, w2, eyer, eyebp, eyebn,
              onesr, onescol, attn_t, dyn, stat)
    nc.compile()
    return nc


def _emit(nc, tc, xq, xk, xv, wq, wk, wv, w1, w2, eyer, eyebp, eyebn,
          onesr, onescol, attn_t, dyn, stat):
    from contextlib import ExitStack
    ctx = ExitStack()
    with ctx:
        consts = ctx.enter_context(tc.tile_pool(name="consts", bufs=1))
        w1_sb = [consts.tile([128, D_MODEL], F32R, name=f"w1_{kc}") for kc in range(KC)]
        w2_sb = [consts.tile([128, D_MODEL], F32R, name=f"w2_{kc}") for kc in range(KC)]
        for kc in range(KC):
            nc.sync.dma_start(out=w1_sb[kc], in_=w1.ap()[kc * 128:(kc + 1) * 128, :])
            nc.sync.dma_start(out=w2_sb[kc], in_=w2.ap()[kc * 128:(kc + 1) * 128, :])
        eyer_sb = consts.tile([128, 128], F32R, name="eyer_sb")
        eyebp_sb = consts.tile([128, 128], BF16, name="eyebp_sb")
        eyebn_sb = consts.tile([128, 128], BF16, name="eyebn_sb")
        nc.sync.dma_start(out=eyer_sb, in_=eyer.ap())
        nc.sync.dma_start(out=eyebp_sb, in_=eyebp.ap())
        nc.sync.dma_start(out=eyebn_sb, in_=eyebn.ap())
        ones1 = consts.tile([1, 128], F32, name="ones1")
        nc.sync.dma_start(out=ones1, in_=onesr.ap())
        eps_sb = consts.tile([128, 1], F32, name="eps_sb")
        nc.vector.memset(eps_sb, EPS)

        # persistent activation layouts (fp32r)
        persist = ctx.enter_context(tc.tile_pool(name="persist", bufs=1))
        qhT = [persist.tile([128, L], F32R, name=f"qhT_{p}") for p in range(4)]
        khT = [persist.tile([128, L], F32R, name=f"khT_{p}") for p in range(4)]
        vhT = [persist.tile([128, L], F32R, name=f"vhT_{p}") for p in range(4)]
        vhp = [persist.tile([128, N_HEAD, D_V + 1], F32R, name=f"vhp_{c}")
               for c in range(LC)]
        otp = [persist.tile([128, L], F32R, name=f"otp_{p}") for p in range(4)]

        # ---------------- phase 1: LN + transpose + projections ----------
        with tc.tile_pool(name="p1_w", bufs=1) as p1_w, \
             tc.tile_pool(name="p1_xnt", bufs=1) as p1_xnt, \
             tc.tile_pool(name="p1_x", bufs=9) as p1_x, \
             tc.tile_pool(name="p1_sb", bufs=3) as p1_sb, \
             tc.tile_pool(name="p1_st", bufs=2) as p1_st, \
             tc.tile_pool(name="p1_ps_tr", bufs=4, space="PSUM") as p1_ps_tr, \
             tc.tile_pool(name="p1_ps_pj", bufs=3, space="PSUM") as p1_ps_pj:

            wq_sb = [p1_w.tile([128, D_IN], F32R, name=f"wq_{kc}") for kc in range(KC)]
            wk_sb = [p1_w.tile([128, D_IN], F32R, name=f"wk_{kc}") for kc in range(KC)]
            wv_sb = [p1_w.tile([128, D_IN], F32R, name=f"wv_{kc}") for kc in range(KC)]
            for kc in range(KC):
                nc.sync.dma_start(out=wq_sb[kc], in_=wq.ap()[kc * 128:(kc + 1) * 128, :])
                nc.sync.dma_start(out=wk_sb[kc], in_=wk.ap()[kc * 128:(kc + 1) * 128, :])
                nc.sync.dma_start(out=wv_sb[kc], in_=wv.ap()[kc * 128:(kc + 1) * 128, :])

            xnT = {}
            for t in range(3):
                for kc in range(KC):
                    xnT[(t, kc)] = p1_xnt.tile([128, L], F32R, name=f"xnT_{t}_{kc}")

            for t, x_in in enumerate([xq, xk, xv]):
                xts = []
                mvall = p1_st.tile([128, 2 * LC], F32, name="mvall", tag="mvall")
                for c in range(LC):
                    xt = p1_x.tile([128, D_IN], F32, name="xt", tag="xt")
                    nc.sync.dma_start(out=xt, in_=x_in.ap()[c * 128:(c + 1) * 128, :])
                    xts.append(xt)
                    stt = p1_sb.tile([128, 6], F32, name="stt", tag="stt")
                    nc.vector.bn_stats(out=stt, in_=xt)
                    nc.vector.bn_aggr(out=mvall[:, 2 * c:2 * c + 2], in_=stt)
                # batched rstd: sig = sqrt(var + eps); rs = 1/sig
                sg = p1_st.tile([128, LC], F32, name="sg", tag="sg")
                var_view = mvall.rearrange("p (c two) -> p c two", two=2)[:, :, 1]
                nc.scalar.activation(sg, var_view, AF.Sqrt, bias=eps_sb)
                rs = p1_st.tile([128, LC], F32, name="rs", tag="rs")
                nc.vector.reciprocal(out=rs, in_=sg)
                ps_tr = {}
                for c in range(LC):
                    xn = p1_sb.tile([128, D_IN], F32R, name="xn", tag="xn")
                    nc.vector.tensor_scalar(
                        out=xn, in0=xts[c], scalar1=mvall[:, 2 * c:2 * c + 1],
                        scalar2=rs[:, c:c + 1],
                        op0=ALU.subtract, op1=ALU.mult)
                    half = c // 4
                    for dc in range(KC):
                        key = (dc, half)
                        if key not in ps_tr:
                            ps_tr[key] = p1_ps_tr.tile([128, 512], F32R,
                                                       name="ps_tr", tag="ps_tr")
                        cc = c % 4
                        nc.tensor.transpose(
                            ps_tr[key][:, cc * 128:(cc + 1) * 128],
                            xn[:, dc * 128:(dc + 1) * 128], eyer_sb)
                        if cc == 3:
                            nc.scalar.copy(
                                out=xnT[(t, dc)][:, half * 512:(half + 1) * 512],
                                in_=ps_tr[key])
                            del ps_tr[key]

            # projections: qhT/khT/vhT head-pair [d, l] tiles (fp32r)
            for src, (wt, dstT) in enumerate(
                    ((wq_sb, qhT), (wk_sb, khT), (wv_sb, vhT))):
                for p in range(4):
                    for lh in range(2):
                        pj = p1_ps_pj.tile([128, 512], F32, name="pj", tag="pj")
                        for kc in range(KC):
                            nc.tensor.matmul(
                                pj, wt[kc][:, p * 128:(p + 1) * 128],
                                xnT[(src, kc)][:, lh * 512:(lh + 1) * 512],
                                start=(kc == 0), stop=(kc == KC - 1))
                        nc.scalar.copy(out=dstT[p][:, lh * 512:(lh + 1) * 512],
                                       in_=pj)
            # static = vh_all @ w_fc2 (runs early; fills phase-1 PE idle)
            for c in range(LC):
                ps2 = p1_ps_pj.tile([128, D_MODEL], F32, name="ps2", tag="pj")
                for p in range(4):
                    nc.tensor.matmul(ps2, vhT[p][:, c * 128:(c + 1) * 128],
                                     w2_sb[p], start=(p == 0), stop=(p == 3))
                sb2 = p1_sb.tile([128, D_MODEL], F32, name="sb2", tag="sb2")
                nc.scalar.copy(out=sb2, in_=ps2)
                nc.sync.dma_start(out=stat.ap()[c * 128:(c + 1) * 128, :], in_=sb2)

            # vh natural [l, d] with ones column
            for c in range(LC):
                pj = p1_ps_pj.tile([128, 512], F32, name="pjv", tag="pj")
                for kc in range(KC):
                    nc.tensor.matmul(
                        pj, xnT[(2, kc)][:, c * 128:(c + 1) * 128], wv_sb[kc],
                        start=(kc == 0), stop=(kc == KC - 1))
                nc.vector.tensor_copy(
                    out=vhp[c][:, :, 0:D_V],
                    in_=pj.rearrange("p (h d) -> p h d", h=N_HEAD))
                nc.sync.dma_start(out=vhp[c][:, :, D_V], in_=onescol.ap())

        # ---------------- phase 2: attention per head-pair ---------------
        # et tiles are [128 k, 1024] = (head-in-pair 2, q' 512) for one
        # q-half; normalization is per (pair, q-half, k-chunk): DVE tiles
        # read Zinv-broadcast from PSUM, gpsimd tiles from an SBUF copy.
        with tc.tile_pool(name="p2_et", bufs=13) as p2_et, \
             tc.tile_pool(name="p2_z", bufs=1) as p2_z, \
             tc.tile_pool(name="p2_zb", bufs=2) as p2_zb, \
             tc.tile_pool(name="ps_s", bufs=2, space="PSUM") as ps_s_pool, \
             tc.tile_pool(name="ps_ot", bufs=1, space="PSUM") as ps_ot_pool, \
             tc.tile_pool(name="ps_zb", bufs=1, space="PSUM") as ps_zb_pool:

            for pr in range(4):
                hA = 2 * pr
                for qh in range(2):
                    qs = slice(qh * 512, (qh + 1) * 512)
                    ps_ot = ps_ot_pool.tile([D_V + 1, L], F32, name="ps_ot",
                                            tag="ps_ot")
                    ets = []
                    for kc in range(LC):
                        ps_s = ps_s_pool.tile([128, L], F32, name="ps_s",
                                              tag="ps_s")
                        diag_here = (qh * 512 <= kc * 128 < (qh + 1) * 512)
                        for hh in range(2):  # head in pair; PE row groups
                            o = 64 * hh
                            nc.tensor.matmul(
                                ps_s[:, hh * 512:(hh + 1) * 512],
                                khT[pr][o:o + 64, kc * 128:(kc + 1) * 128],
                                qhT[pr][o:o + 64, qs],
                                start=True, stop=not diag_here)
                        if diag_here:
                            d0 = kc * 128 - qh * 512
                            for hh in range(2):
                                nc.tensor.matmul(
                                    ps_s[:, hh * 512 + d0:hh * 512 + d0 + 128],
                                    eyebp_sb, eyebn_sb, start=False, stop=True)
                        et = p2_et.tile([128, L], F32R, name="et", tag="et")
                        nc.scalar.activation(et, ps_s, AF.Exp)
                        ets.append(et)
                        for hh in range(2):
                            nc.tensor.matmul(
                                ps_ot[:, hh * 512:(hh + 1) * 512],
                                vhp[kc][:, hA + hh, :],
                                et[:, hh * 512:(hh + 1) * 512],
                                start=(kc == 0), stop=(kc == LC - 1))
                    zrow = p2_z.tile([1, L], F32, name="zrow", tag="zrow")
                    nc.vector.tensor_copy(out=zrow, in_=ps_ot[D_V:D_V + 1, :])
                    zinv = p2_z.tile([1, L], F32, name="zinv", tag="zinv")
                    nc.vector.reciprocal_approx_fast(out=zinv, in_=zrow)
                    ps_zb = ps_zb_pool.tile([128, L], F32, name="ps_zb",
                                            tag="ps_zb")
                    for hh in range(2):
                        nc.tensor.matmul(ps_zb[:, hh * 512:(hh + 1) * 512],
                                         ones1, zinv[:, hh * 512:(hh + 1) * 512],
                                         start=True, stop=True)
                    zb_sb = p2_zb.tile([128, L], F32, name="zb_sb", tag="zb")
                    nc.scalar.copy(out=zb_sb, in_=ps_zb)
                    # O^T rows: head A -> otp rows 0-63, head B -> rows 64-127
                    for hh in range(2):
                        o = 64 * hh
                        nc.scalar.copy(out=otp[pr][o:o + 64, qs],
                                       in_=ps_ot[0:D_V, hh * 512:(hh + 1) * 512])
                        nc.vector.tensor_mul(
                            out=otp[pr][o:o + 64, qs],
                            in0=otp[pr][o:o + 64, qs],
                            in1=zb_sb[o:o + 64, hh * 512:(hh + 1) * 512])
                    # normalized attn^T -> HBM (512 KB per DMA)
                    for kc in range(LC):
                        et = ets[kc]
                        eng = nc.gpsimd if kc < ATTN_TT_ON_GPSIMD else nc.vector
                        zsrc = zb_sb if kc < ATTN_TT_ON_GPSIMD else ps_zb
                        with nc.allow_low_precision(reason="fp32r attn tiles"):
                            eng.tensor_mul(out=et, in0=et, in1=zsrc)
                        nc.sync.dma_start(
                            out=attn_t.ap()[pr, qh, kc],
                            in_=et.bitcast(F32).rearrange("p (h q) -> p h q",
                                                          h=2))

        # ---------------- phase 3: dynamic / static ----------------------
        with tc.tile_pool(name="p3_sb", bufs=2) as p3_sb, \
             tc.tile_pool(name="p3_ps", bufs=4, space="PSUM") as p3_ps:
            for c in range(LC):
                pd = p3_ps.tile([128, D_MODEL], F32, name="pd", tag="pd")
                for p in range(4):
                    nc.tensor.matmul(pd, otp[p][:, c * 128:(c + 1) * 128],
                                     w1_sb[p], start=(p == 0), stop=(p == 3))
                db = p3_sb.tile([128, D_MODEL], F32, name="db", tag="db")
                nc.scalar.copy(out=db, in_=pd)
                nc.sync.dma_start(out=dyn.ap()[c * 128:(c + 1) * 128, :], in_=db)



def _get_nc():
    if "nc" not in _CACHED:
        _CACHED["nc"] = _build_nc()
    return _CACHED["nc"]


def kernel(q, k, v, w_q, w_k, w_v, w_fc1, w_fc2,
           ln1_g, ln1_b, ln2_g, ln2_b, ln3_g, ln3_b):
    import ml_dtypes

    q = np.asarray(q, dtype=np.float32)
    k = np.asarray(k, dtype=np.float32)
    v = np.asarray(v, dtype=np.float32)

    # fold LN gain into the projection weights; 1/sqrt(d_k) into w_q
    wq_s = (np.asarray(w_q) * np.asarray(ln1_g)[:, None] / np.sqrt(D_K)).astype(np.float32)
    wk_s = (np.asarray(w_k) * np.asarray(ln2_g)[:, None]).astype(np.float32)
    wv_s = (np.asarray(w_v) * np.asarray(ln3_g)[:, None]).astype(np.float32)
    for name, bb in (("ln1_b", ln1_b), ("ln2_b", ln2_b), ("ln3_b", ln3_b)):
        if np.any(np.asarray(bb)):
            raise NotImplementedError(f"nonzero {name} not supported")

    eyep = np.eye(128, dtype=np.float32)
    eyebp = eyep.astype(ml_dtypes.bfloat16)
    eyebn = (eyep * NEG_BIG).astype(ml_dtypes.bfloat16)

    nc = _get_nc()
    common = {
        "wq": np.ascontiguousarray(wq_s), "wk": np.ascontiguousarray(wk_s),
        "wv": np.ascontiguousarray(wv_s),
        "w1": np.ascontiguousarray(np.asarray(w_fc1, dtype=np.float32)),
        "w2": np.ascontiguousarray(np.asarray(w_fc2, dtype=np.float32)),
        "eyer": eyep, "eyebp": eyebp, "eyebn": eyebn,
        "onesr": np.ones((1, 128), dtype=np.float32),
        "onescol": np.ones((128, N_HEAD), dtype=np.float32),
    }
    in_maps = [
        {"xq": np.ascontiguousarray(q[b]), "xk": np.ascontiguousarray(k[b]),
         "xv": np.ascontiguousarray(v[b]), **common}
        for b in range(N_CORES)
    ]
    res = bass_utils.run_bass_kernel_spmd(
        nc, in_maps, core_ids=list(range(N_CORES)), **_CACHED.get("run_kwargs", {}))
    _CACHED["last_result"] = res

    dynamic = np.stack([res.results[b]["dyn"] for b in range(N_CORES)])
    static = np.stack([res.results[b]["stat"] for b in range(N_CORES)])
    # attn_t[b] is [pr, qh, kc, k', hh, q']; attn_flat[(2pr+hh)*B+b, 512qh+q',
    # 128kc+k'] = attn_t[b][pr, qh, kc, k', hh, q']
    at = np.stack([res.results[b]["attn_t"] for b in range(N_CORES)], axis=1)
    # at: [pr, b, qh, kc, k', hh, q'] -> [pr, hh, b, qh, q', kc, k']
    attn_flat = np.ascontiguousarray(at.transpose(0, 5, 1, 2, 6, 3, 4)).reshape(
        N_HEAD * B, L, L)
    return dynamic, static, attn_flat
